# revision 22
# baseline (speedup 1.0000x reference)
"""ArcFace-style per-class loss kernel for 8 Trainium2 NeuronCores.

Math (algebraically exact reduction of the reference):
  Xn_i  = X_i / ||X_i||
  sums_c = sum_{i: l_i=c} Xn_i               [C, D] segment sum
  counts_c = |{i: l_i=c}|   (host bincount, shipped as an input)
  loss_c = (S_c * lse_seg_c - ||sums_c||) / max(counts_c, 1)
    with S_c = colsum_c/||sums_c||, colsum_c = sum_d sums_c[d]
  Because rows are unit-norm, lse_i = log(D + 1/2 + sum_d Xn_id) + O(1e-5)
  (2nd-order Taylor of logsumexp using sum_d Xn^2 = 1), so
  lse_seg_c = K*counts_c + colsum_c/(D+1/2),  K = log(D+1/2).

Sharding: rows are routed (on host) to the core owning their label octant
(128 classes per core via balanced binning), so every per-class reduction
is fully local to one core — no collectives.  X ships as fp16 (half the
HBM traffic of fp32; final rel err ~3e-4, well under the 2e-2 gate).

Per 128-row tile: row sum-of-squares via fused square+accumulate spread
across ACT (Square), GpSimd (stt) and DVE (stt) so no engine exceeds the
DMA-stream budget; rnorm = reciprocal_approx_fast(sqrt(ss)) (~18 bits);
scaled one-hot = (iota==label)*rnorm in one fused DVE tensor_scalar; PE
accumulates sums (one-hotT @ X) into PSUM across all tiles.  Padded rows
have label -1 (zero one-hot column) and X = 0.
"""

import sys

if "/opt/trn_rl_repo" not in sys.path:
    sys.path.insert(0, "/opt/trn_rl_repo")

import math

import numpy as np

import concourse.bass as bass  # noqa: F401
import concourse.tile as tile
from concourse import bacc, mybir
from concourse.bass_utils import run_bass_kernel_spmd

# Problem constants (hardcoded per spec: N=131072, D=512, C=1024, 8 cores)
N_ROWS = 131072
D = 512
C = 1024
NCORES = 8
CLOC = C // NCORES  # 128 classes per core

# Classes are assigned to cores by balanced greedy bin-packing (128 classes
# per core, near-equal row totals), so per-core rows ~ N/8 = 16384 +- ~16.
# Capacity 16512 = 8 full groups of 2048 rows + one 1-tile (128-row) tail.
CAP = 16512
P = 128  # partitions / rows per tile
NT = CAP // P  # 129 tiles
G = 16  # tiles per full group (one DMA per group)
NG = 8  # full groups
G_TAIL = 1  # tiles in the tail group
# per full group: how many row-tiles' sum-of-squares go to ACT; the rest
# go to DVE (which also builds every one-hot).  GpSimd cannot run
# TensorScalarPtr (walrus birverifier rejects it), so it only helps with
# semaphores/DMA.
N_ACT = 9
N_POOL = 0
NCHUNK = 4  # DMA chunks per full group


def set_config(g=16, n_act=10, n_pool=0, nchunk=4):
    global G, NG, N_ACT, N_POOL, NCHUNK
    G = g
    NG = (CAP - G_TAIL * P) // (P * g)
    N_ACT = n_act
    N_POOL = n_pool
    NCHUNK = nchunk
    assert NG * G * P + G_TAIL * P == CAP


K_CONST = math.log(D + 0.5)
INV_D5 = 1.0 / (D + 0.5)

F32 = mybir.dt.float32
F16 = mybir.dt.float16


def _group_engines(gg, n_act, n_pool):
    """Interleaved engine assignment for the gg ss-tiles of one group."""
    n_dve = gg - n_act - n_pool
    assert n_dve >= 0
    quota = {"A": n_act, "P": n_pool, "D": n_dve}
    used = {"A": 0, "P": 0, "D": 0}
    order = []
    for j in range(gg):
        # largest remaining fraction first
        best = max(
            ("A", "P", "D"),
            key=lambda e: (quota[e] - used[e]) / max(quota[e], 1e-9)
            if quota[e]
            else -1,
        )
        used[best] += 1
        order.append(best)
    return order


def build_nc():
    nc = bacc.Bacc(None, target_bir_lowering=False)

    # (pad rows are set to e0 = [1,0,...,0] on the host, so ss >= 1 for
    # every row and no epsilon guard is needed anywhere in the group path)

    x_ext = nc.declare_dram_parameter("x", [NG, P, G, D], F16, isOutput=False)
    xt_ext = nc.declare_dram_parameter("xt", [P, G_TAIL, D], F16, isOutput=False)
    lab_ext = nc.declare_dram_parameter("lab", [P, NT], F32, isOutput=False)
    iota_ext = nc.declare_dram_parameter("iota", [P, CLOC], F16, isOutput=False)
    cnt_ext = nc.declare_dram_parameter("cnt", [P, 1], F32, isOutput=False)
    out_ext = nc.declare_dram_parameter("out", [P, 1], F32, isOutput=True)

    AF = mybir.ActivationFunctionType
    OP = mybir.AluOpType

    with tile.TileContext(nc) as tc:
        with (
            tc.tile_pool(name="xpool", bufs=5) as xpool,
            tc.tile_pool(name="ohpool", bufs=18) as ohpool,
            tc.tile_pool(name="small", bufs=8) as small,
            tc.tile_pool(name="singles", bufs=1) as singles,
            tc.tile_pool(name="psum", bufs=1, space="PSUM") as psum,
        ):
            # keep the sync ring free for the X stream: side inputs load
            # via the scalar-engine HWDGE ring
            lab_sb = singles.tile([P, NT], F32)
            nc.scalar.dma_start(out=lab_sb[:], in_=lab_ext[:, :])
            iota_sb = singles.tile([P, CLOC], F16)
            nc.scalar.dma_start(out=iota_sb[:], in_=iota_ext[:, :])
            cnt_sb = singles.tile([P, 1], F32)
            nc.scalar.dma_start(out=cnt_sb[:], in_=cnt_ext[:, :])

            # prefetch the sqrt activation table while the first DMAs run
            warm = singles.tile([P, 1], F32)
            nc.vector.memset(warm[:], 1.0)
            nc.scalar.activation(out=warm[:], in_=warm[:], func=AF.Sqrt)

            # 1/max(cnt,1) depends only on the cnt input: compute it up
            # front so the epilogue's critical path is shorter
            cc_early = singles.tile([P, 1], F32)
            nc.vector.tensor_scalar_max(cc_early[:], cnt_sb[:], 1.0)
            ic_early = singles.tile([P, 1], F32)
            nc.vector.reciprocal_approx_fast(out=ic_early[:], in_=cc_early[:])

            psum_sums = psum.tile([P, D], F32)  # one full bank
            act_scratch = psum.tile([P, D], F32)  # ACT Square dump
            dve_scratch = singles.tile([P, D], F16)  # DVE stt dump (2-byte)

            def emit_ss(g, src_ap, gg, n_act):
                """DMA the group's X and produce its per-row rnorm."""
                xg = xpool.tile([P, gg, D], F16, tag="xg", name=f"xg{g}")
                nchunk = NCHUNK if gg >= NCHUNK else 1
                cs = gg // nchunk
                for ci in range(nchunk):
                    nc.sync.dma_start(
                        out=xg[:, ci * cs : (ci + 1) * cs],
                        in_=src_ap[:, ci * cs : (ci + 1) * cs],
                    )

                # per-row sum of squares, split ACT / DVE
                ssg = small.tile([P, gg], F32, tag="ssg", name=f"ssg{g}")
                for j, eng in enumerate(_group_engines(gg, n_act, 0)):
                    if eng == "A":
                        nc.scalar.activation(
                            out=act_scratch[:],
                            in_=xg[:, j],
                            func=AF.Square,
                            accum_out=ssg[:, j : j + 1],
                        )
                    else:
                        nc.vector.scalar_tensor_tensor(
                            out=dve_scratch[:],
                            in0=xg[:, j],
                            scalar=1.0,
                            in1=xg[:, j],
                            op0=OP.mult,
                            op1=OP.mult,
                            accum_out=ssg[:, j : j + 1],
                        )

                # rnorm = 1/sqrt(ss) via sqrt + fast reciprocal (ss >= 1:
                # pad rows are e0 on host, real rows are chi^2(512)-sized)
                sqg = small.tile([P, gg], F32, tag="sqg", name=f"sqg{g}")
                nc.scalar.activation(out=sqg[:], in_=ssg[:], func=AF.Sqrt)
                rn = small.tile([P, gg], F32, tag="rn", name=f"rn{g}")
                nc.vector.reciprocal_approx_fast(out=rn[:], in_=sqg[:])
                return xg, rn

            def emit_oh_mm(t_base, xg, rn, gg):
                """Build scaled one-hots and run the PE accumulation."""
                for j in range(gg):
                    t = t_base + j
                    oh = ohpool.tile([P, CLOC], F16, tag="oh", name=f"oh{t}")
                    nc.vector.tensor_scalar(
                        oh[:],
                        iota_sb[:],
                        lab_sb[:, t : t + 1],
                        rn[:, j : j + 1],
                        OP.is_equal,
                        OP.mult,
                    )
                    nc.tensor.matmul(
                        psum_sums[:],
                        lhsT=oh[:],
                        rhs=xg[:, j],
                        start=(t == 0),
                        stop=(t == NT - 1),
                    )

            # Software-pipelined emission: each group's sum-of-squares is
            # emitted one group AHEAD of its one-hot + matmul phase, so
            # ACT's sqrt for group g+1 never queues behind DVE's 16 one-hot
            # builds for group g (in-order engine queues).  The 1-tile tail
            # group goes LAST so the drain after the final sqrt gate is a
            # single one-hot, not sixteen.
            groups = [
                (g, g * G, x_ext[g], G, N_ACT) for g in range(NG)
            ]
            groups += [(NG, NG * G, xt_ext[:, :, :], G_TAIL, 1)]
            pend = None  # (t_base, xg, rn, gg) awaiting oh+mm
            for g, t_base, src_ap, gg, n_act in groups:
                xg, rn = emit_ss(g, src_ap, gg, n_act)
                if pend is not None:
                    emit_oh_mm(*pend)
                pend = (t_base, xg, rn, gg)
            emit_oh_mm(*pend)

            # ---- epilogue: per-class loss from sums/counts ----
            # sumsq on ACT (Square+accum) and colsum on DVE run in parallel;
            # each reads PSUM via a single non-scalar input (IBVF027).
            junk2 = singles.tile([P, D], F32)
            sumsq = singles.tile([P, 1], F32)
            nc.scalar.activation(
                out=junk2[:], in_=psum_sums[:], func=AF.Square,
                accum_out=sumsq[:],
            )
            junk = singles.tile([P, D], F32)
            colsum = singles.tile([P, 1], F32)
            nc.vector.tensor_scalar(
                junk[:], psum_sums[:], 1.0, 0.0, OP.mult, OP.add,
                accum_out=colsum[:],
            )

            _ep_n = [0]

            def newt():
                _ep_n[0] += 1
                return singles.tile(
                    [P, 1], F32, name=f"ep{_ep_n[0]}", tag=f"ep{_ep_n[0]}"
                )

            s0 = newt()
            nc.vector.tensor_scalar_max(s0[:], sumsq[:], 1e-20)
            sq2 = newt()
            nc.scalar.activation(out=sq2[:], in_=s0[:], func=AF.Sqrt)
            ri = newt()
            nc.vector.reciprocal_approx_fast(out=ri[:], in_=sq2[:])
            mask = newt()
            nc.vector.tensor_scalar(mask[:], sumsq[:], 1e-12, None, OP.is_gt)
            sm = newt()
            nc.vector.tensor_mul(sm[:], colsum[:], ri[:])
            S = newt()
            nc.vector.tensor_mul(S[:], sm[:], mask[:])
            l2 = newt()
            nc.vector.tensor_scalar_mul(l2[:], colsum[:], INV_D5)
            lseg = newt()
            nc.vector.scalar_tensor_tensor(
                out=lseg[:], in0=cnt_sb[:], scalar=K_CONST, in1=l2[:],
                op0=OP.mult, op1=OP.add,
            )
            aa = newt()
            nc.vector.tensor_mul(aa[:], S[:], lseg[:])
            bb = newt()
            nc.vector.tensor_mul(bb[:], sq2[:], mask[:])
            num = newt()
            nc.vector.scalar_tensor_tensor(
                out=num[:], in0=bb[:], scalar=-1.0, in1=aa[:],
                op0=OP.mult, op1=OP.add,
            )
            loss = newt()
            nc.vector.tensor_mul(loss[:], num[:], ic_early[:])

            # scalar-engine HWDGE ring: independent FIFO, so this tiny store
            # does not queue behind the X-stream DMA completion receipts
            nc.scalar.dma_start(out=out_ext[:, :], in_=loss[:])

    nc.compile()
    return nc


def assign_classes(labels):
    """Greedy balanced partition: 128 classes per core, near-equal row totals.
    Returns (owner_of_cls [C], pos_of_cls [C], cls_at [NCORES, CLOC], rows)."""
    counts = np.bincount(labels, minlength=C)
    order = np.argsort(-counts, kind="stable")
    bin_rows = np.zeros(NCORES, dtype=np.int64)
    bin_n = np.zeros(NCORES, dtype=np.int64)
    owner_of_cls = np.empty(C, dtype=np.int64)
    pos_of_cls = np.empty(C, dtype=np.int64)
    cls_at = np.empty((NCORES, CLOC), dtype=np.int64)
    for cidx in order:
        open_bins = np.flatnonzero(bin_n < CLOC)
        k = open_bins[np.argmin(bin_rows[open_bins])]
        owner_of_cls[cidx] = k
        pos_of_cls[cidx] = bin_n[k]
        cls_at[k, bin_n[k]] = cidx
        bin_n[k] += 1
        bin_rows[k] += counts[cidx]
    return owner_of_cls, pos_of_cls, cls_at, bin_rows, counts


def make_in_maps(logits, labels):
    """Host-side sharding: route each row to the core owning its (balanced)
    class bin; lay X out fp16 so each partition's per-group data is
    contiguous in DRAM."""
    logits = np.asarray(logits, dtype=np.float32)
    labels = np.asarray(labels).astype(np.int64)
    owner_of_cls, pos_of_cls, cls_at, bin_rows, counts = assign_classes(labels)
    assert bin_rows.max() <= CAP, f"max shard {bin_rows.max()} > capacity {CAP}"
    owner = owner_of_cls[labels]
    local = pos_of_cls[labels]
    in_maps = []
    iota_tile = np.ascontiguousarray(
        np.broadcast_to(
            np.arange(CLOC, dtype=np.float16), (P, CLOC)
        )
    )
    for k in range(NCORES):
        idx = np.flatnonzero(owner == k)
        nk = idx.size
        xs = np.zeros((CAP, D), dtype=np.float16)
        xs[:nk] = logits[idx]
        xs[nk:, 0] = 1.0  # pad rows are e0 so row sum-of-squares is 1
        # full groups: row (g*G + j)*P + p -> x4[g, p, j, :]
        x4 = np.ascontiguousarray(
            xs[: NG * G * P].reshape(NG, G, P, D).transpose(0, 2, 1, 3)
        )
        xt = np.ascontiguousarray(
            xs[NG * G * P :].reshape(G_TAIL, P, D).transpose(1, 0, 2)
        )
        ll = np.full((CAP,), -1.0, dtype=np.float32)
        ll[:nk] = local[idx].astype(np.float32)
        # device tile order: full groups first, tail tile last
        lab2d = np.ascontiguousarray(ll.reshape(NT, P).T)  # [p, t]
        cnt2d = counts[cls_at[k]].astype(np.float32).reshape(P, 1)
        in_maps.append(
            {"x": x4, "xt": xt, "lab": lab2d, "iota": iota_tile, "cnt": cnt2d}
        )
    return in_maps, cls_at


_NC_CACHE = {}


def get_nc():
    if "nc" not in _NC_CACHE:
        _NC_CACHE["nc"] = build_nc()
    return _NC_CACHE["nc"]


def run(logits, labels, num_classes, trace=False, **spmd_kwargs):
    assert int(num_classes) == C
    nc = get_nc()
    in_maps, cls_at = make_in_maps(logits, labels)
    res = run_bass_kernel_spmd(
        nc, in_maps, core_ids=list(range(NCORES)), trace=trace, **spmd_kwargs
    )
    out = np.empty((C,), dtype=np.float32)
    for k in range(NCORES):
        out[cls_at[k]] = res.results[k]["out"].ravel()
    return out, res


def kernel(logits, labels, num_classes):
    out, _ = run(logits, labels, num_classes)
    return out


# revision 23
# speedup vs baseline: 1.0092x; 1.0092x over previous
"""ArcFace-style per-class loss kernel for 8 Trainium2 NeuronCores.

Math (algebraically exact reduction of the reference):
  Xn_i  = X_i / ||X_i||
  sums_c = sum_{i: l_i=c} Xn_i               [C, D] segment sum
  counts_c = |{i: l_i=c}|   (host bincount, shipped as an input)
  loss_c = (S_c * lse_seg_c - ||sums_c||) / max(counts_c, 1)
    with S_c = colsum_c/||sums_c||, colsum_c = sum_d sums_c[d]
  Because rows are unit-norm, lse_i = log(D + 1/2 + sum_d Xn_id) + O(1e-5)
  (2nd-order Taylor of logsumexp using sum_d Xn^2 = 1), so
  lse_seg_c = K*counts_c + colsum_c/(D+1/2),  K = log(D+1/2).

Sharding: rows are routed (on host) to the core owning their label octant
(128 classes per core via balanced binning), so every per-class reduction
is fully local to one core — no collectives.  X ships as fp16 (half the
HBM traffic of fp32; final rel err ~3e-4, well under the 2e-2 gate).

Per 128-row tile: row sum-of-squares via fused square+accumulate spread
across ACT (Square), GpSimd (stt) and DVE (stt) so no engine exceeds the
DMA-stream budget; rnorm = reciprocal_approx_fast(sqrt(ss)) (~18 bits);
scaled one-hot = (iota==label)*rnorm in one fused DVE tensor_scalar; PE
accumulates sums (one-hotT @ X) into PSUM across all tiles.  Padded rows
have label -1 (zero one-hot column) and X = 0.
"""

import sys

if "/opt/trn_rl_repo" not in sys.path:
    sys.path.insert(0, "/opt/trn_rl_repo")

import math

import numpy as np

import concourse.bass as bass  # noqa: F401
import concourse.tile as tile
from concourse import bacc, mybir
from concourse.bass_utils import run_bass_kernel_spmd

# Problem constants (hardcoded per spec: N=131072, D=512, C=1024, 8 cores)
N_ROWS = 131072
D = 512
C = 1024
NCORES = 8
CLOC = C // NCORES  # 128 classes per core

# Classes are assigned to cores by balanced greedy bin-packing (128 classes
# per core, near-equal row totals), so per-core rows ~ N/8 = 16384 +- ~16.
# Capacity 16512 = 8 full groups of 2048 rows + one 1-tile (128-row) tail.
CAP = 16512
P = 128  # partitions / rows per tile
NT = CAP // P  # 129 tiles
G = 16  # tiles per full group (one DMA per group)
NG = 8  # full groups
G_TAIL = 1  # tiles in the tail group
# per full group: how many row-tiles' sum-of-squares go to ACT; the rest
# go to DVE (which also builds every one-hot).  GpSimd cannot run
# TensorScalarPtr (walrus birverifier rejects it), so it only helps with
# semaphores/DMA.
N_ACT = 9
N_POOL = 0
NCHUNK = 4  # DMA chunks per full group


def set_config(g=16, n_act=10, n_pool=0, nchunk=4):
    global G, NG, N_ACT, N_POOL, NCHUNK
    G = g
    NG = (CAP - G_TAIL * P) // (P * g)
    N_ACT = n_act
    N_POOL = n_pool
    NCHUNK = nchunk
    assert NG * G * P + G_TAIL * P == CAP


K_CONST = math.log(D + 0.5)
INV_D5 = 1.0 / (D + 0.5)

F32 = mybir.dt.float32
F16 = mybir.dt.float16


def _group_engines(gg, n_act, n_pool):
    """Interleaved engine assignment for the gg ss-tiles of one group."""
    n_dve = gg - n_act - n_pool
    assert n_dve >= 0
    quota = {"A": n_act, "P": n_pool, "D": n_dve}
    used = {"A": 0, "P": 0, "D": 0}
    order = []
    for j in range(gg):
        # largest remaining fraction first
        best = max(
            ("A", "P", "D"),
            key=lambda e: (quota[e] - used[e]) / max(quota[e], 1e-9)
            if quota[e]
            else -1,
        )
        used[best] += 1
        order.append(best)
    return order


def build_nc():
    nc = bacc.Bacc(None, target_bir_lowering=False)

    # (pad rows are set to e0 = [1,0,...,0] on the host, so ss >= 1 for
    # every row and no epsilon guard is needed anywhere in the group path)

    x_ext = nc.declare_dram_parameter("x", [NG, P, G, D], F16, isOutput=False)
    xt_ext = nc.declare_dram_parameter("xt", [P, G_TAIL, D], F16, isOutput=False)
    lab_ext = nc.declare_dram_parameter("lab", [P, NT], F32, isOutput=False)
    iota_ext = nc.declare_dram_parameter("iota", [P, CLOC], F16, isOutput=False)
    cnt_ext = nc.declare_dram_parameter("cnt", [P, 1], F32, isOutput=False)
    out_ext = nc.declare_dram_parameter("out", [P, 1], F32, isOutput=True)

    AF = mybir.ActivationFunctionType
    OP = mybir.AluOpType

    with tile.TileContext(nc) as tc:
        with (
            tc.tile_pool(name="xpool", bufs=5) as xpool,
            tc.tile_pool(name="ohpool", bufs=18) as ohpool,
            tc.tile_pool(name="small", bufs=8) as small,
            tc.tile_pool(name="singles", bufs=1) as singles,
            tc.tile_pool(name="psum", bufs=1, space="PSUM") as psum,
        ):
            # keep the sync ring free for the X stream: side inputs load
            # via the scalar-engine HWDGE ring
            lab_sb = singles.tile([P, NT], F32)
            nc.scalar.dma_start(out=lab_sb[:], in_=lab_ext[:, :])
            iota_sb = singles.tile([P, CLOC], F16)
            nc.scalar.dma_start(out=iota_sb[:], in_=iota_ext[:, :])
            cnt_sb = singles.tile([P, 1], F32)
            nc.scalar.dma_start(out=cnt_sb[:], in_=cnt_ext[:, :])

            # prefetch the sqrt activation table while the first DMAs run
            warm = singles.tile([P, 1], F32)
            nc.vector.memset(warm[:], 1.0)
            nc.scalar.activation(out=warm[:], in_=warm[:], func=AF.Sqrt)

            # 1/max(cnt,1) depends only on the cnt input: compute it up
            # front so the epilogue's critical path is shorter
            cc_early = singles.tile([P, 1], F32)
            nc.vector.tensor_scalar_max(cc_early[:], cnt_sb[:], 1.0)
            ic_early = singles.tile([P, 1], F32)
            nc.vector.reciprocal_approx_fast(out=ic_early[:], in_=cc_early[:])

            psum_sums = psum.tile([P, D], F32)  # one full bank
            act_scratch = psum.tile([P, D], F32)  # ACT Square dump
            dve_scratch = singles.tile([P, D], F16)  # DVE stt dump (2-byte)

            def emit_ss_tile(xg, ssg, j, eng):
                if eng == "A":
                    nc.scalar.activation(
                        out=act_scratch[:],
                        in_=xg[:, j],
                        func=AF.Square,
                        accum_out=ssg[:, j : j + 1],
                    )
                else:
                    nc.vector.scalar_tensor_tensor(
                        out=dve_scratch[:],
                        in0=xg[:, j],
                        scalar=1.0,
                        in1=xg[:, j],
                        op0=OP.mult,
                        op1=OP.mult,
                        accum_out=ssg[:, j : j + 1],
                    )

            def emit_oh_mm_tile(t_base, xg, rn, j):
                t = t_base + j
                oh = ohpool.tile([P, CLOC], F16, tag="oh", name=f"oh{t}")
                nc.vector.tensor_scalar(
                    oh[:],
                    iota_sb[:],
                    lab_sb[:, t : t + 1],
                    rn[:, j : j + 1],
                    OP.is_equal,
                    OP.mult,
                )
                nc.tensor.matmul(
                    psum_sums[:],
                    lhsT=oh[:],
                    rhs=xg[:, j],
                    start=(t == 0),
                    stop=(t == NT - 1),
                )

            # Software-pipelined, chunk-interleaved emission.  Group g+1's
            # sum-of-squares tiles are emitted between group g's one-hot
            # builds — with the (data-ready) one-hots FIRST in each round so
            # DVE's in-order queue never head-of-line blocks on a
            # sum-of-squares tile whose DMA chunk hasn't landed yet.  The
            # 1-tile tail group goes last so the final drain is short.
            groups = [(g, g * G, x_ext[g], G, N_ACT) for g in range(NG)]
            groups += [(NG, NG * G, xt_ext[:, :, :], G_TAIL, 1)]
            pend = None  # (t_base, xg, rn, gg) awaiting oh+mm
            for g, t_base, src_ap, gg, n_act in groups:
                xg = xpool.tile([P, gg, D], F16, tag="xg", name=f"xg{g}")
                nchunk = NCHUNK if gg >= NCHUNK else 1
                cs = gg // nchunk
                for ci in range(nchunk):
                    nc.sync.dma_start(
                        out=xg[:, ci * cs : (ci + 1) * cs],
                        in_=src_ap[:, ci * cs : (ci + 1) * cs],
                    )
                ssg = small.tile([P, gg], F32, tag="ssg", name=f"ssg{g}")
                engines = _group_engines(gg, n_act, 0)
                pg = pend[3] if pend is not None else 0  # prev tiles to emit
                for ci in range(nchunk):
                    if pend is not None:
                        lo = pg * ci // nchunk
                        hi = pg * (ci + 1) // nchunk
                        for j in range(lo, hi):
                            emit_oh_mm_tile(pend[0], pend[1], pend[2], j)
                    for j in range(ci * cs, (ci + 1) * cs):
                        emit_ss_tile(xg, ssg, j, engines[j])
                # rnorm = 1/sqrt(ss) via sqrt + fast reciprocal (ss >= 1:
                # pad rows are e0 on host, real rows are chi^2(512)-sized)
                sqg = small.tile([P, gg], F32, tag="sqg", name=f"sqg{g}")
                nc.scalar.activation(out=sqg[:], in_=ssg[:], func=AF.Sqrt)
                rn = small.tile([P, gg], F32, tag="rn", name=f"rn{g}")
                nc.vector.reciprocal_approx_fast(out=rn[:], in_=sqg[:])
                pend = (t_base, xg, rn, gg)
            for j in range(pend[3]):
                emit_oh_mm_tile(pend[0], pend[1], pend[2], j)

            # ---- epilogue: per-class loss from sums/counts ----
            # sumsq on ACT (Square+accum) and colsum on DVE run in parallel;
            # each reads PSUM via a single non-scalar input (IBVF027).
            junk2 = singles.tile([P, D], F32)
            sumsq = singles.tile([P, 1], F32)
            nc.scalar.activation(
                out=junk2[:], in_=psum_sums[:], func=AF.Square,
                accum_out=sumsq[:],
            )
            junk = singles.tile([P, D], F32)
            colsum = singles.tile([P, 1], F32)
            nc.vector.tensor_scalar(
                junk[:], psum_sums[:], 1.0, 0.0, OP.mult, OP.add,
                accum_out=colsum[:],
            )

            _ep_n = [0]

            def newt():
                _ep_n[0] += 1
                return singles.tile(
                    [P, 1], F32, name=f"ep{_ep_n[0]}", tag=f"ep{_ep_n[0]}"
                )

            s0 = newt()
            nc.vector.tensor_scalar_max(s0[:], sumsq[:], 1e-20)
            sq2 = newt()
            nc.scalar.activation(out=sq2[:], in_=s0[:], func=AF.Sqrt)
            ri = newt()
            nc.vector.reciprocal_approx_fast(out=ri[:], in_=sq2[:])
            mask = newt()
            nc.vector.tensor_scalar(mask[:], sumsq[:], 1e-12, None, OP.is_gt)
            sm = newt()
            nc.vector.tensor_mul(sm[:], colsum[:], ri[:])
            S = newt()
            nc.vector.tensor_mul(S[:], sm[:], mask[:])
            l2 = newt()
            nc.vector.tensor_scalar_mul(l2[:], colsum[:], INV_D5)
            lseg = newt()
            nc.vector.scalar_tensor_tensor(
                out=lseg[:], in0=cnt_sb[:], scalar=K_CONST, in1=l2[:],
                op0=OP.mult, op1=OP.add,
            )
            aa = newt()
            nc.vector.tensor_mul(aa[:], S[:], lseg[:])
            bb = newt()
            nc.vector.tensor_mul(bb[:], sq2[:], mask[:])
            num = newt()
            nc.vector.scalar_tensor_tensor(
                out=num[:], in0=bb[:], scalar=-1.0, in1=aa[:],
                op0=OP.mult, op1=OP.add,
            )
            loss = newt()
            nc.vector.tensor_mul(loss[:], num[:], ic_early[:])

            # scalar-engine HWDGE ring: independent FIFO, so this tiny store
            # does not queue behind the X-stream DMA completion receipts
            nc.scalar.dma_start(out=out_ext[:, :], in_=loss[:])

    nc.compile()
    return nc


def assign_classes(labels):
    """Greedy balanced partition: 128 classes per core, near-equal row totals.
    Returns (owner_of_cls [C], pos_of_cls [C], cls_at [NCORES, CLOC], rows)."""
    counts = np.bincount(labels, minlength=C)
    order = np.argsort(-counts, kind="stable")
    bin_rows = np.zeros(NCORES, dtype=np.int64)
    bin_n = np.zeros(NCORES, dtype=np.int64)
    owner_of_cls = np.empty(C, dtype=np.int64)
    pos_of_cls = np.empty(C, dtype=np.int64)
    cls_at = np.empty((NCORES, CLOC), dtype=np.int64)
    for cidx in order:
        open_bins = np.flatnonzero(bin_n < CLOC)
        k = open_bins[np.argmin(bin_rows[open_bins])]
        owner_of_cls[cidx] = k
        pos_of_cls[cidx] = bin_n[k]
        cls_at[k, bin_n[k]] = cidx
        bin_n[k] += 1
        bin_rows[k] += counts[cidx]
    return owner_of_cls, pos_of_cls, cls_at, bin_rows, counts


def make_in_maps(logits, labels):
    """Host-side sharding: route each row to the core owning its (balanced)
    class bin; lay X out fp16 so each partition's per-group data is
    contiguous in DRAM."""
    logits = np.asarray(logits, dtype=np.float32)
    labels = np.asarray(labels).astype(np.int64)
    owner_of_cls, pos_of_cls, cls_at, bin_rows, counts = assign_classes(labels)
    assert bin_rows.max() <= CAP, f"max shard {bin_rows.max()} > capacity {CAP}"
    owner = owner_of_cls[labels]
    local = pos_of_cls[labels]
    in_maps = []
    iota_tile = np.ascontiguousarray(
        np.broadcast_to(
            np.arange(CLOC, dtype=np.float16), (P, CLOC)
        )
    )
    for k in range(NCORES):
        idx = np.flatnonzero(owner == k)
        nk = idx.size
        xs = np.zeros((CAP, D), dtype=np.float16)
        xs[:nk] = logits[idx]
        xs[nk:, 0] = 1.0  # pad rows are e0 so row sum-of-squares is 1
        # full groups: row (g*G + j)*P + p -> x4[g, p, j, :]
        x4 = np.ascontiguousarray(
            xs[: NG * G * P].reshape(NG, G, P, D).transpose(0, 2, 1, 3)
        )
        xt = np.ascontiguousarray(
            xs[NG * G * P :].reshape(G_TAIL, P, D).transpose(1, 0, 2)
        )
        ll = np.full((CAP,), -1.0, dtype=np.float32)
        ll[:nk] = local[idx].astype(np.float32)
        # device tile order: full groups first, tail tile last
        lab2d = np.ascontiguousarray(ll.reshape(NT, P).T)  # [p, t]
        cnt2d = counts[cls_at[k]].astype(np.float32).reshape(P, 1)
        in_maps.append(
            {"x": x4, "xt": xt, "lab": lab2d, "iota": iota_tile, "cnt": cnt2d}
        )
    return in_maps, cls_at


_NC_CACHE = {}


def get_nc():
    if "nc" not in _NC_CACHE:
        _NC_CACHE["nc"] = build_nc()
    return _NC_CACHE["nc"]


def run(logits, labels, num_classes, trace=False, **spmd_kwargs):
    assert int(num_classes) == C
    nc = get_nc()
    in_maps, cls_at = make_in_maps(logits, labels)
    res = run_bass_kernel_spmd(
        nc, in_maps, core_ids=list(range(NCORES)), trace=trace, **spmd_kwargs
    )
    out = np.empty((C,), dtype=np.float32)
    for k in range(NCORES):
        out[cls_at[k]] = res.results[k]["out"].ravel()
    return out, res


def kernel(logits, labels, num_classes):
    out, _ = run(logits, labels, num_classes)
    return out
